# revision 10
# baseline (speedup 1.0000x reference)
"""Trainium2 Bass kernel for nn_Mlp_70798240907434 (content-gated conv MLP).

Sharding: 8 cores = 4 batches x 2 spatial halves (rows 0-47 / 48-95).
Each core computes the full layer-1 (1x1 dynamic conv + gelu) for its batch
(needed for the global max-pools feeding the dynamic-kernel generation), then
its half of the 3x3 dynamic conv (layer 2). The half offset enters only
through dynamic (register) rhs offsets derived from partition_id, so all 8
cores share one SPMD program. No collectives.

Self-contained: hardcodes shapes from the problem spec.
"""

import contextlib

import numpy as np

import concourse.bass as bass
import concourse.mybir as mybir
import concourse.tile as tile
from concourse import bacc
from concourse.bass_utils import run_bass_kernel_spmd

F32 = mybir.dt.float32
F32R = mybir.dt.float32r

B, CIN, CHID, COUT, H, W = 4, 64, 256, 64, 96, 96
S = H * W                      # 9216
HALF_ROWS = H // 2             # 48
HALF = HALF_ROWS * W           # 4608

# padded h layout: (1+96+1) rows x (1+96+1) cols, flat, +1 front spare +3 back
PW = W + 2                     # 98
HB = 1                         # front spare (tap base can be -1)
HPF = HB + PW * PW + 3         # 9608

# layer-1 spatial tiling: 4 rows per tile (384 cols), 24 tiles
L1_ROWS = 4
L1_NT = H // L1_ROWS           # 24
L1_N = L1_ROWS * W             # 384
XCHUNK_ROWS = 12               # x loaded in 8 chunks of 12 rows
NXCH = H // XCHUNK_ROWS        # 8

# layer-2 spatial tiling (own half): 5-row tiles in padded coords
L2_ROWS = 5
L2_TILES = [(t0, min(L2_ROWS, HALF_ROWS - t0)) for t0 in range(0, HALF_ROWS, L2_ROWS)]


def _build():
    nc = bacc.Bacc()

    # ---- DRAM parameters (per-core) ----
    x64 = nc.declare_dram_parameter("x64", [CIN, S], F32R, isOutput=False)
    x128 = nc.declare_dram_parameter("x128", [CIN, S], F32R, isOutput=False)
    w1t = nc.declare_dram_parameter("w1t", [CIN, CHID], F32, isOutput=False)
    bd1 = nc.declare_dram_parameter("bd1", [CIN, CHID], F32, isOutput=False)
    ce1v = nc.declare_dram_parameter("ce1v", [CIN, 1], F32, isOutput=False)
    gd1v = nc.declare_dram_parameter("gd1v", [CIN, 1], F32, isOutput=False)
    gd21v = nc.declare_dram_parameter("gd21v", [CIN, 1], F32, isOutput=False)
    ones1_64 = nc.declare_dram_parameter("ones1_64", [1, CIN], F32, isOutput=False)
    ident = nc.declare_dram_parameter("ident", [128, 128], F32, isOutput=False)
    w2t = nc.declare_dram_parameter("w2t", [CHID, 9 * COUT], F32, isOutput=False)
    bd2 = nc.declare_dram_parameter("bd2", [CHID, COUT], F32, isOutput=False)
    cewt = nc.declare_dram_parameter("cewt", [9, 5], F32, isOutput=False)
    gdt = nc.declare_dram_parameter("gdt", [5, 9], F32, isOutput=False)
    gd2x = nc.declare_dram_parameter("gd2x", [5, 9 * COUT], F32, isOutput=False)
    ones5 = nc.declare_dram_parameter("ones5", [5, 1], F32, isOutput=False)
    ones1_128 = nc.declare_dram_parameter("ones1_128", [1, 128], F32, isOutput=False)
    y = nc.declare_dram_parameter("y", [COUT, HALF], F32, isOutput=True)

    with tile.TileContext(nc) as tc, contextlib.ExitStack() as ctx:
        consts = ctx.enter_context(tc.tile_pool(name="consts", bufs=1))
        big = ctx.enter_context(tc.tile_pool(name="big", bufs=1))
        small = ctx.enter_context(tc.tile_pool(name="small", bufs=2))

        # ---- load small constants ----
        w1t_sb = consts.tile([CIN, CHID], F32, tag="w1t")
        bd1_sb = consts.tile([CIN, CHID], F32, tag="bd1")
        ce1_sb = consts.tile([CIN, 1], F32, tag="ce1")
        gd1_sb = consts.tile([CIN, 1], F32, tag="gd1")
        gd21_sb = consts.tile([CIN, 1], F32, tag="gd21")
        on64_sb = consts.tile([1, CIN], F32, tag="on64")
        id_sb = consts.tile([128, 128], F32, tag="ident")
        w2t_sb = [consts.tile([128, 9 * COUT], F32, tag=f"w2t{t}", name=f"w2t{t}") for t in range(2)]
        bd2_sb = [consts.tile([128, COUT], F32, tag=f"bd2{t}", name=f"bd2{t}") for t in range(2)]
        cewt_sb = consts.tile([9, 5], F32, tag="cewt")
        gdt_sb = consts.tile([5, 9], F32, tag="gdt")
        gd2x_sb = consts.tile([5, 9 * COUT], F32, tag="gd2x")
        on5_sb = consts.tile([5, 1], F32, tag="on5")
        on128_sb = consts.tile([1, 128], F32, tag="on128")
        for t, d in [
            (w1t_sb, w1t), (bd1_sb, bd1), (ce1_sb, ce1v), (gd1_sb, gd1v),
            (gd21_sb, gd21v), (on64_sb, ones1_64), (id_sb, ident),
            (cewt_sb, cewt), (gdt_sb, gdt), (gd2x_sb, gd2x),
            (on5_sb, ones5), (on128_sb, ones1_128),
        ]:
            nc.sync.dma_start(t[:], d[:])
        for t in range(2):
            nc.sync.dma_start(w2t_sb[t][:], w2t[t * 128:(t + 1) * 128, :])
            nc.sync.dma_start(bd2_sb[t][:], bd2[t * 128:(t + 1) * 128, :])

        # ---- x loads ----
        # x64: [64, 9216] (c partitions) in 8 row-chunks for the L1 matmuls
        xch = [consts.tile([CIN, XCHUNK_ROWS * W], F32R, tag=f"xch{k}", name=f"xch{k}")
               for k in range(NXCH)]
        for k in range(NXCH):
            nc.sync.dma_start(
                xch[k][:], x64[:, k * XCHUNK_ROWS * W:(k + 1) * XCHUNK_ROWS * W])
        # x128: [128, 4608] (p = half*64 + c) for the global-max reduce
        x128_sb = big.tile([128, HALF], F32R, tag="x128")
        csz = HALF // NXCH
        for k in range(NXCH):
            src = bass.AP(tensor=x128[:].tensor, offset=k * csz,
                          ap=[[HALF, 2], [S, CIN], [1, csz]])
            nc.sync.dma_start(x128_sb[:, k * csz:(k + 1) * csz], src)

        # ---- h_pad tiles (padded gelu output), zero the pad regions ----
        hpad = [big.tile([128, HPF], F32R, tag=f"hpad{t}", name=f"hpad{t}") for t in range(2)]
        for t in range(2):
            hp = hpad[t][:].bitcast(F32)
            # front spare + top pad row
            nc.vector.memset(hp[:, 0:HB + PW], 0.0)
            # bottom pad row + back spare
            nc.vector.memset(hp[:, HB + 97 * PW:HPF], 0.0)
            # left/right pad cols of rows 1..96: offset HB+PW, [(PW,96),(97,2)]
            colpad = bass.AP(
                tensor=hp.tensor, offset=HB + PW,
                ap=[list(hp.ap[0]), [PW, 96], [97, 2]])
            nc.vector.memset(colpad, 0.0)

        # ---- gl1: global per-channel max of x ----
        xmax8 = small.tile([128, NXCH], F32, tag="xmax8")
        for k in range(NXCH):
            nc.vector.reduce_max(xmax8[:, k:k + 1], x128_sb[:, k * csz:(k + 1) * csz],
                                 axis=mybir.AxisListType.X)
        xmax1 = small.tile([128, 1], F32, tag="xmax1")
        nc.vector.reduce_max(xmax1[:], xmax8[:], axis=mybir.AxisListType.X)
        xhi = small.tile([CIN, 1], F32, tag="xhi")
        nc.sync.dma_start(xhi[:], xmax1[CIN:128, :])
        gl1_sb = small.tile([CIN, 1], F32, tag="gl1")
        nc.vector.tensor_tensor(out=gl1_sb[:], in0=xmax1[0:CIN, :], in1=xhi[:],
                                op=mybir.AluOpType.max)

        # ---- dyn1 generation ----
        rce1_sb = small.tile([CIN, 1], F32, tag="rce1")
        nc.scalar.activation(rce1_sb[:], gl1_sb[:],
                             mybir.ActivationFunctionType.Relu, scale=ce1_sb[:])
        outc_sb = small.tile([CIN, 1], F32, tag="outc")
        nc.scalar.activation(outc_sb[:], rce1_sb[:],
                             mybir.ActivationFunctionType.Copy, scale=gd1_sb[:])
        ps_a = tc.alloc_tile_pool(name="ps_a", bufs=2, space="PSUM")
        ocp0_ps = ps_a.tile([1, CHID], F32, tag="a", name="ocp0_ps")
        nc.tensor.matmul(ocp0_ps[:], rce1_sb[:], bd1_sb[:], start=True, stop=True)
        rocp1_sb = small.tile([1, CHID], F32, tag="rocp1")
        nc.scalar.activation(rocp1_sb[:], ocp0_ps[:],
                             mybir.ActivationFunctionType.Relu)
        sig1_ps = ps_a.tile([CIN, CHID], F32, tag="a", name="sig1_ps")
        nc.tensor.matmul(sig1_ps[:], on64_sb[:], rocp1_sb[:], start=True, stop=True)
        sg1_sb = small.tile([CIN, CHID], F32, tag="sg1")
        nc.scalar.activation(sg1_sb[:], sig1_ps[:],
                             mybir.ActivationFunctionType.Sigmoid,
                             bias=outc_sb[:], scale=gd21_sb[:])
        dyn1_sb = small.tile([CIN, CHID], F32R, tag="dyn1")
        nc.vector.tensor_mul(dyn1_sb[:], sg1_sb[:], w1t_sb[:])

        # ---- layer 1: z = dyn1.T @ x ; h = gelu(z) -> hpad; pool stage A ----
        ps_a.release()
        ps_big = tc.alloc_tile_pool(name="ps_big", bufs=4, space="PSUM")
        stageA = [big.tile([128, H * 3], F32, tag=f"stA{t}", name=f"stA{t}") for t in range(2)]
        for j in range(L1_NT):          # 4-row tiles over the full image
            ch, sub = j // 3, j % 3
            rhs = xch[ch][:, sub * L1_N:(sub + 1) * L1_N]
            for m in range(2):          # oc tile
                z_ps = ps_big.tile([128, L1_N], F32, tag="z")
                nc.tensor.matmul(z_ps[:], dyn1_sb[:, m * 128:(m + 1) * 128], rhs,
                                 start=True, stop=True)
                # gelu eviction into padded layout (rows 4j..4j+3)
                dst = bass.AP(
                    tensor=hpad[m][:].tensor,
                    offset=HB + (4 * j + 1) * PW + 1,
                    ap=[list(hpad[m][:].ap[0]), [PW, L1_ROWS], [1, W]])
                src = bass.AP(tensor=z_ps[:].tensor, offset=z_ps[:].offset,
                              ap=[list(z_ps[:].ap[0]), [W, L1_ROWS], [1, W]])
                nc.scalar.activation(dst, src, mybir.ActivationFunctionType.Gelu)
                # pool stage A from post-gelu h: per-row 32-col maxes
                # (gelu is NOT monotone, so the pool must read h, not z)
                hpf32 = hpad[m][:].bitcast(F32)
                pin = bass.AP(tensor=hpf32.tensor,
                              offset=HB + (4 * j + 1) * PW + 1,
                              ap=[list(hpf32.ap[0]), [PW, L1_ROWS], [32, 3], [1, 32]])
                nc.vector.reduce_max(
                    stageA[m][:, 4 * j * 3:(4 * j + L1_ROWS) * 3], pin,
                    axis=mybir.AxisListType.X)

        # ---- pool stage B -> gl2 [128, 9] per ctile ----
        gl2_sb = [small.tile([128, 9], F32, tag=f"gl2_{t}", name=f"gl2_{t}") for t in range(2)]
        for t in range(2):
            sA = stageA[t][:]
            pin = bass.AP(tensor=sA.tensor, offset=sA.offset,
                          ap=[list(sA.ap[0]), [96, 3], [1, 3], [3, 32]])
            nc.vector.reduce_max(gl2_sb[t][:], pin, axis=mybir.AxisListType.X)

        # ---- dyn2 generation ----
        ps_big.release()
        ps_c = tc.alloc_tile_pool(name="ps_c", bufs=2, space="PSUM")
        # gl2T [9, 256]
        gl2t_sb = small.tile([9, CHID], F32, tag="gl2t")
        for t in range(2):
            tp_ps = ps_c.tile([9, 128], F32, tag="c", name="tp_ps")
            nc.tensor.transpose(tp_ps[:], gl2_sb[t][:], id_sb[:])
            nc.scalar.activation(gl2t_sb[:, t * 128:(t + 1) * 128], tp_ps[:],
                                 mybir.ActivationFunctionType.Copy)
        # ce2T = cewt.T @ gl2T : [5, 256]
        ce2t_ps = ps_c.tile([5, CHID], F32, tag="c2", name="ce2t_ps", bufs=1)
        nc.tensor.matmul(ce2t_ps[:], cewt_sb[:], gl2t_sb[:], start=True, stop=True)
        rce2t_sb = small.tile([5, CHID], F32, tag="rce2t")
        nc.scalar.activation(rce2t_sb[:], ce2t_ps[:],
                             mybir.ActivationFunctionType.Relu)
        # ce2 (c-partition): [128, 5] per ctile ; then ocp0T accum [5, 64]
        ocp0t_ps = ps_c.tile([5, COUT], F32, tag="c3", name="ocp0t_ps", bufs=1)
        rce2c_sb = [small.tile([128, 5], F32, tag=f"rce2c{t}", name=f"rce2c{t}") for t in range(2)]
        for t in range(2):
            c_ps = ps_c.tile([128, 5], F32, tag="c", name="c_ps")
            nc.tensor.matmul(c_ps[:], gl2t_sb[:, t * 128:(t + 1) * 128], cewt_sb[:],
                             start=True, stop=True)
            nc.scalar.activation(rce2c_sb[t][:], c_ps[:],
                                 mybir.ActivationFunctionType.Relu)
        for t in range(2):
            nc.tensor.matmul(ocp0t_ps[:], rce2c_sb[t][:], bd2_sb[t][:],
                             start=(t == 0), stop=(t == 1))
        rocp2_sb = small.tile([5, COUT], F32, tag="rocp2")
        nc.scalar.activation(rocp2_sb[:], ocp0t_ps[:],
                             mybir.ActivationFunctionType.Relu)
        # gr = rocp2 (bcast over k) * gd2x : [5, 576]
        gr_sb = small.tile([5, 9 * COUT], F32, tag="gr")
        rocp_b = bass.AP(tensor=rocp2_sb[:].tensor, offset=rocp2_sb[:].offset,
                         ap=[list(rocp2_sb[:].ap[0]), [0, 9], [1, COUT]])
        nc.vector.tensor_mul(gr_sb[:], rocp_b, gd2x_sb[:])
        # ocprow [1, 576] = ones5.T @ gr (N=576 -> split 512+64)
        ocprow_ps = ps_c.tile([1, 9 * COUT], F32, tag="c2", name="ocprow_ps", bufs=1)
        nc.tensor.matmul(ocprow_ps[:, 0:512], on5_sb[:], gr_sb[:, 0:512],
                         start=True, stop=True)
        nc.tensor.matmul(ocprow_ps[:, 512:576], on5_sb[:], gr_sb[:, 512:576],
                         start=True, stop=True)
        ocprow_sb = small.tile([1, 9 * COUT], F32, tag="ocprow_sb")
        nc.scalar.activation(ocprow_sb[:], ocprow_ps[:],
                             mybir.ActivationFunctionType.Copy)
        # outTT [128, 9] per ctile
        outtt_sb = [small.tile([128, 9], F32, tag=f"outtt{t}", name=f"outtt{t}") for t in range(2)]
        for t in range(2):
            o_ps = ps_c.tile([128, 9], F32, tag="c", name="o_ps")
            nc.tensor.matmul(o_ps[:], rce2t_sb[:, t * 128:(t + 1) * 128], gdt_sb[:],
                             start=True, stop=True)
            nc.scalar.activation(outtt_sb[t][:], o_ps[:],
                                 mybir.ActivationFunctionType.Copy)
        # S = bcast(ocprow) + bcast(outTT); sigmoid; * w2t -> dyn2 [128, 576] x2
        dyn2_sb = [small.tile([128, 9 * COUT], F32R, tag=f"dyn2_{t}", name=f"dyn2_{t}")
                   for t in range(2)]
        for t in range(2):
            bc_ps = ps_c.tile([128, 9 * COUT], F32, tag="c4", name="bc_ps", bufs=1)
            nc.tensor.matmul(bc_ps[:, 0:512], on128_sb[:], ocprow_sb[:, 0:512],
                             start=True, stop=True)
            nc.tensor.matmul(bc_ps[:, 512:576], on128_sb[:], ocprow_sb[:, 512:576],
                             start=True, stop=True)
            s_sb = small.tile([128, 9 * COUT], F32, tag="s_sb")
            ott = outtt_sb[t][:]
            ott_b = bass.AP(tensor=ott.tensor, offset=ott.offset,
                            ap=[list(ott.ap[0]), [1, 9], [0, COUT]])
            nc.vector.tensor_add(s_sb[:], bc_ps[:], ott_b)
            sg_sb = small.tile([128, 9 * COUT], F32, tag="sg2")
            nc.scalar.activation(sg_sb[:], s_sb[:],
                                 mybir.ActivationFunctionType.Sigmoid)
            nc.vector.tensor_mul(dyn2_sb[t][:], sg_sb[:], w2t_sb[t][:])

        # ---- layer 2: 3x3 dynamic conv over own half (dynamic row offset) ----
        ps_c.release()
        ps_y = tc.alloc_tile_pool(name="ps_y", bufs=4, space="PSUM")
        pid = nc.partition_id()
        off = nc.snap((pid % 2) * (HALF_ROWS * PW), min_val=0,
                      max_val=HALF_ROWS * PW)
        y_sb = big.tile([COUT, HALF], F32, tag="ysb")
        for t0, R in L2_TILES:
            n = PW * R
            yp = ps_y.tile([COUT, n], F32, tag="yp")
            k = 0
            for di in range(3):
                for dj in range(3):
                    base = HB + (t0 + di) * PW + dj - 1
                    for t in range(2):
                        nc.tensor.matmul(
                            yp[:],
                            dyn2_sb[t][:, (3 * di + dj) * COUT:
                                       (3 * di + dj + 1) * COUT],
                            hpad[t][:, bass.ds(off + base, n)],
                            start=(k == 0), stop=(k == 17))
                        k += 1
            src = bass.AP(tensor=yp[:].tensor, offset=yp[:].offset + 1,
                          ap=[list(yp[:].ap[0]), [PW, R], [1, W]])
            nc.vector.tensor_copy(y_sb[:, t0 * W:(t0 + R) * W], src)
            nc.sync.dma_start(y[:, t0 * W:(t0 + R) * W],
                              y_sb[:, t0 * W:(t0 + R) * W])
        ps_y.release()

    nc.finalize()
    return nc


_CACHE = {}


def _get_nc():
    if "nc" not in _CACHE:
        _CACHE["nc"] = _build()
    return _CACHE["nc"]


def _host_weights(fc1_weight, fc1_ce, fc1_gd, fc1_gd2, fc1_ci,
                  fc2_weight, fc2_ce, fc2_gd, fc2_gd2, fc2_ci):
    f = np.float32
    w1 = fc1_weight.reshape(CHID, CIN).astype(f)
    # bd1[c, p*32+o] = fc1_ci[o, c%8] where p = c//8
    bd1 = np.zeros((CIN, CHID), f)
    for c in range(CIN):
        p, g = c // 8, c % 8
        bd1[c, p * 32:(p + 1) * 32] = fc1_ci[:, g]
    # bd2[c, p*2+o] = fc2_ci[o, c%8] where p = c//8
    bd2 = np.zeros((CHID, COUT), f)
    for c in range(CHID):
        p, g = c // 8, c % 8
        bd2[c, p * 2:p * 2 + 2] = fc2_ci[:, g]
    w2t = np.ascontiguousarray(
        fc2_weight.reshape(COUT, CHID, 9).transpose(1, 2, 0).reshape(CHID, 9 * COUT)
    ).astype(f)
    gd2x = np.ascontiguousarray(
        np.repeat(fc2_gd2.T, COUT, axis=1)).astype(f)     # [5, 9*64]
    return {
        "w1t": np.ascontiguousarray(w1.T).astype(f),
        "bd1": bd1,
        "ce1v": np.full((CIN, 1), fc1_ce[0, 0], f),
        "gd1v": np.full((CIN, 1), fc1_gd[0, 0], f),
        "gd21v": np.full((CIN, 1), fc1_gd2[0, 0], f),
        "ones1_64": np.ones((1, CIN), f),
        "ident": np.eye(128, dtype=f),
        "w2t": w2t,
        "bd2": bd2,
        "cewt": np.ascontiguousarray(fc2_ce.T).astype(f),
        "gdt": np.ascontiguousarray(fc2_gd.T).astype(f),
        "gd2x": gd2x,
        "ones5": np.ones((5, 1), f),
        "ones1_128": np.ones((1, 128), f),
    }


def run(inputs, trace=False):
    nc = _get_nc()
    shared = _host_weights(
        inputs["fc1_weight"], inputs["fc1_ce"], inputs["fc1_gd"],
        inputs["fc1_gd2"], inputs["fc1_ci"], inputs["fc2_weight"],
        inputs["fc2_ce"], inputs["fc2_gd"], inputs["fc2_gd2"], inputs["fc2_ci"])
    x = np.asarray(inputs["x"], np.float32)
    in_maps = []
    for core in range(8):
        bi = core // 2
        xb = np.ascontiguousarray(x[bi].reshape(CIN, S))
        in_maps.append({"x64": xb, "x128": xb, **shared})
    res = run_bass_kernel_spmd(nc, in_maps, list(range(8)), trace=trace)
    out = np.empty((B, COUT, H, W), np.float32)
    for core in range(8):
        bi, half = core // 2, core % 2
        out[bi, :, half * HALF_ROWS:(half + 1) * HALF_ROWS, :] = (
            res.results[core]["y"].reshape(COUT, HALF_ROWS, W))
    return out, res


def kernel(**inputs):
    out, _ = run(inputs, trace=False)
    return out


# revision 12
# speedup vs baseline: 1.4476x; 1.4476x over previous
"""Trainium2 Bass kernel for nn_Mlp_70798240907434 (content-gated conv MLP).

Sharding: 8 cores = 4 batches x 2 spatial halves (rows 0-47 / 48-95).
Each core computes the full layer-1 (1x1 dynamic conv + gelu) for its batch
(needed for the global max-pools feeding the dynamic-kernel generation), then
its half of the 3x3 dynamic conv (layer 2). The half offset enters only
through dynamic (register) rhs offsets derived from partition_id, so all 8
cores share one SPMD program. No collectives.

Self-contained: hardcodes shapes from the problem spec.
"""

import contextlib

import numpy as np

import concourse.bass as bass
import concourse.mybir as mybir
import concourse.tile as tile
from concourse import bacc
from concourse.bass_utils import run_bass_kernel_spmd

F32 = mybir.dt.float32
F32R = mybir.dt.float32r

B, CIN, CHID, COUT, H, W = 4, 64, 256, 64, 96, 96
S = H * W                      # 9216
HALF_ROWS = H // 2             # 48
HALF = HALF_ROWS * W           # 4608

# padded h layout: (1+96+1) rows x (1+96+1) cols, flat, +1 front spare +3 back
PW = W + 2                     # 98
HB = 1                         # front spare (tap base can be -1)
HPF = HB + PW * PW + 3         # 9608

# layer-1 spatial tiling: 8 rows per tile, split as 2 x 384-col matmuls into
# a 2-bank psum tile; 12 tiles
L1_ROWS = 8
L1_NT = H // L1_ROWS           # 12
L1_N = 384                     # cols per matmul (4 rows)
XCHUNK_ROWS = 16               # x loaded in 6 chunks of 16 rows
NXCH = H // XCHUNK_ROWS        # 6

# layer-2 spatial tiling (own half): 5-row tiles in padded coords
L2_ROWS = 5
L2_TILES = [(t0, min(L2_ROWS, HALF_ROWS - t0)) for t0 in range(0, HALF_ROWS, L2_ROWS)]


def _build():
    nc = bacc.Bacc()

    # ---- DRAM parameters (per-core) ----
    x64 = nc.declare_dram_parameter("x64", [CIN, S], F32R, isOutput=False)
    x128 = nc.declare_dram_parameter("x128", [CIN, S], F32R, isOutput=False)
    w1t = nc.declare_dram_parameter("w1t", [CIN, CHID], F32, isOutput=False)
    bd1 = nc.declare_dram_parameter("bd1", [CIN, CHID], F32, isOutput=False)
    ce1v = nc.declare_dram_parameter("ce1v", [CIN, 1], F32, isOutput=False)
    gd1v = nc.declare_dram_parameter("gd1v", [CIN, 1], F32, isOutput=False)
    gd21v = nc.declare_dram_parameter("gd21v", [CIN, 1], F32, isOutput=False)
    ones1_64 = nc.declare_dram_parameter("ones1_64", [1, CIN], F32, isOutput=False)
    ident = nc.declare_dram_parameter("ident", [128, 128], F32, isOutput=False)
    w2t = nc.declare_dram_parameter("w2t", [CHID, 9 * COUT], F32, isOutput=False)
    bd2 = nc.declare_dram_parameter("bd2", [CHID, COUT], F32, isOutput=False)
    cewt = nc.declare_dram_parameter("cewt", [9, 5], F32, isOutput=False)
    gdt = nc.declare_dram_parameter("gdt", [5, 9], F32, isOutput=False)
    gd2x = nc.declare_dram_parameter("gd2x", [5, 9 * COUT], F32, isOutput=False)
    ones5 = nc.declare_dram_parameter("ones5", [5, 1], F32, isOutput=False)
    ones1_128 = nc.declare_dram_parameter("ones1_128", [1, 128], F32, isOutput=False)
    y = nc.declare_dram_parameter("y", [COUT, HALF], F32, isOutput=True)

    with tile.TileContext(nc) as tc, contextlib.ExitStack() as ctx:
        consts = ctx.enter_context(tc.tile_pool(name="consts", bufs=1))
        big = ctx.enter_context(tc.tile_pool(name="big", bufs=1))
        small = ctx.enter_context(tc.tile_pool(name="small", bufs=2))

        # ---- load small constants ----
        w1t_sb = consts.tile([CIN, CHID], F32, tag="w1t")
        bd1_sb = consts.tile([CIN, CHID], F32, tag="bd1")
        ce1_sb = consts.tile([CIN, 1], F32, tag="ce1")
        gd1_sb = consts.tile([CIN, 1], F32, tag="gd1")
        gd21_sb = consts.tile([CIN, 1], F32, tag="gd21")
        on64_sb = consts.tile([1, CIN], F32, tag="on64")
        id_sb = consts.tile([128, 128], F32, tag="ident")
        w2t_sb = [consts.tile([128, 9 * COUT], F32, tag=f"w2t{t}", name=f"w2t{t}") for t in range(2)]
        bd2_sb = [consts.tile([128, COUT], F32, tag=f"bd2{t}", name=f"bd2{t}") for t in range(2)]
        cewt_sb = consts.tile([9, 5], F32, tag="cewt")
        gdt_sb = consts.tile([5, 9], F32, tag="gdt")
        gd2x_sb = consts.tile([5, 9 * COUT], F32, tag="gd2x")
        on5_sb = consts.tile([5, 1], F32, tag="on5")
        on128_sb = consts.tile([1, 128], F32, tag="on128")
        for t, d in [
            (w1t_sb, w1t), (bd1_sb, bd1), (ce1_sb, ce1v), (gd1_sb, gd1v),
            (gd21_sb, gd21v), (on64_sb, ones1_64), (id_sb, ident),
            (cewt_sb, cewt), (gdt_sb, gdt), (gd2x_sb, gd2x),
            (on5_sb, ones5), (on128_sb, ones1_128),
        ]:
            nc.scalar.dma_start(t[:], d[:])
        for t in range(2):
            nc.scalar.dma_start(w2t_sb[t][:], w2t[t * 128:(t + 1) * 128, :])
            nc.scalar.dma_start(bd2_sb[t][:], bd2[t * 128:(t + 1) * 128, :])

        # ---- x loads ----
        # x64: [64, 9216] (c partitions) in 8 row-chunks for the L1 matmuls
        xch = [consts.tile([CIN, XCHUNK_ROWS * W], F32R, tag=f"xch{k}", name=f"xch{k}")
               for k in range(NXCH)]
        for k in range(NXCH):
            nc.sync.dma_start(
                xch[k][:], x64[:, k * XCHUNK_ROWS * W:(k + 1) * XCHUNK_ROWS * W])

        # ---- h_pad tiles (padded gelu output), zero the pad regions ----
        hpad = [big.tile([128, HPF], F32R, tag=f"hpad{t}", name=f"hpad{t}") for t in range(2)]
        for t in range(2):
            hp = hpad[t][:].bitcast(F32)
            # front spare + top pad row
            nc.vector.memset(hp[:, 0:HB + PW], 0.0)
            # bottom pad row + back spare
            nc.vector.memset(hp[:, HB + 97 * PW:HPF], 0.0)
            # left/right pad cols of rows 1..96: offset HB+PW, [(PW,96),(97,2)]
            colpad = bass.AP(
                tensor=hp.tensor, offset=HB + PW,
                ap=[list(hp.ap[0]), [PW, 96], [97, 2]])
            nc.vector.memset(colpad, 0.0)

        # ---- gl1: global per-channel max of x (from the x64 chunks) ----
        xmaxc = small.tile([CIN, NXCH], F32, tag="xmaxc")
        for k in range(NXCH):
            nc.vector.reduce_max(xmaxc[:, k:k + 1], xch[k][:],
                                 axis=mybir.AxisListType.X)
        gl1_sb = small.tile([CIN, 1], F32, tag="gl1")
        nc.vector.reduce_max(gl1_sb[:], xmaxc[:], axis=mybir.AxisListType.X)

        # ---- dyn1 generation ----
        rce1_sb = small.tile([CIN, 1], F32, tag="rce1")
        nc.vector.tensor_scalar(rce1_sb[:], gl1_sb[:], ce1_sb[:], 0.0,
                                mybir.AluOpType.mult, mybir.AluOpType.max)
        outc_sb = small.tile([CIN, 1], F32, tag="outc")
        nc.vector.tensor_scalar_mul(outc_sb[:], rce1_sb[:], gd1_sb[:])
        ps_a = tc.alloc_tile_pool(name="ps_a", bufs=2, space="PSUM")
        ocp0_ps = ps_a.tile([1, CHID], F32, tag="a", name="ocp0_ps")
        nc.tensor.matmul(ocp0_ps[:], rce1_sb[:], bd1_sb[:], start=True, stop=True)
        rocp1_sb = small.tile([1, CHID], F32, tag="rocp1")
        nc.vector.tensor_scalar_max(rocp1_sb[:], ocp0_ps[:], 0.0)
        sig1_ps = ps_a.tile([CIN, CHID], F32, tag="a", name="sig1_ps")
        nc.tensor.matmul(sig1_ps[:], on64_sb[:], rocp1_sb[:], start=True, stop=True)
        sg1_sb = small.tile([CIN, CHID], F32, tag="sg1")
        nc.scalar.activation(sg1_sb[:], sig1_ps[:],
                             mybir.ActivationFunctionType.Sigmoid,
                             bias=outc_sb[:], scale=gd21_sb[:])
        dyn1_sb = small.tile([CIN, CHID], F32R, tag="dyn1")
        nc.vector.tensor_mul(dyn1_sb[:], sg1_sb[:], w1t_sb[:])

        # ---- layer 1: z = dyn1.T @ x ; h = gelu(z) -> hpad; pool stage A ----
        ps_a.release()
        ps_big = tc.alloc_tile_pool(name="ps_big", bufs=3, space="PSUM")
        stageA = [big.tile([128, H * 3], F32, tag=f"stA{t}", name=f"stA{t}") for t in range(2)]
        for j in range(L1_NT):          # 8-row tiles over the full image
            ch, sub = j // 2, j % 2
            for m in range(2):          # oc tile
                z_ps = ps_big.tile([128, 2, 512], F32, tag="z")
                for i in range(2):      # two 4-row matmuls, one psum bank each
                    rhs = xch[ch][:, (sub * 2 + i) * L1_N:(sub * 2 + i + 1) * L1_N]
                    nc.tensor.matmul(z_ps[:, i, 0:L1_N],
                                     dyn1_sb[:, m * 128:(m + 1) * 128], rhs,
                                     start=True, stop=True)
                # gelu eviction into padded layout (rows 8j..8j+7)
                dst = bass.AP(
                    tensor=hpad[m][:].tensor,
                    offset=HB + (8 * j + 1) * PW + 1,
                    ap=[list(hpad[m][:].ap[0]), [PW * 4, 2], [PW, 4], [1, W]])
                src = bass.AP(tensor=z_ps[:].tensor, offset=z_ps[:].offset,
                              ap=[list(z_ps[:].ap[0]), [512, 2], [W, 4], [1, W]])
                nc.scalar.activation(dst, src, mybir.ActivationFunctionType.Gelu)
                # pool stage A from post-gelu h: per-row 32-col maxes
                # (gelu is NOT monotone, so the pool must read h, not z)
                hpf32 = hpad[m][:].bitcast(F32)
                pin = bass.AP(tensor=hpf32.tensor,
                              offset=HB + (8 * j + 1) * PW + 1,
                              ap=[list(hpf32.ap[0]), [PW, L1_ROWS], [32, 3], [1, 32]])
                nc.vector.reduce_max(
                    stageA[m][:, 8 * j * 3:(8 * j + L1_ROWS) * 3], pin,
                    axis=mybir.AxisListType.X)

        # ---- pool stage B -> gl2 [128, 9] per ctile ----
        gl2_sb = [small.tile([128, 9], F32, tag=f"gl2_{t}", name=f"gl2_{t}") for t in range(2)]
        for t in range(2):
            sA = stageA[t][:]
            pin = bass.AP(tensor=sA.tensor, offset=sA.offset,
                          ap=[list(sA.ap[0]), [96, 3], [1, 3], [3, 32]])
            nc.vector.reduce_max(gl2_sb[t][:], pin, axis=mybir.AxisListType.X)

        # ---- dyn2 generation ----
        ps_big.release()
        ps_c = tc.alloc_tile_pool(name="ps_c", bufs=2, space="PSUM")
        # gl2T [9, 256]
        gl2t_sb = small.tile([9, CHID], F32, tag="gl2t")
        for t in range(2):
            tp_ps = ps_c.tile([9, 128], F32, tag="c", name="tp_ps")
            nc.tensor.transpose(tp_ps[:], gl2_sb[t][:], id_sb[:])
            nc.vector.tensor_copy(gl2t_sb[:, t * 128:(t + 1) * 128], tp_ps[:])
        # ce2T = cewt.T @ gl2T : [5, 256]
        ce2t_ps = ps_c.tile([5, CHID], F32, tag="c2", name="ce2t_ps", bufs=1)
        nc.tensor.matmul(ce2t_ps[:], cewt_sb[:], gl2t_sb[:], start=True, stop=True)
        rce2t_sb = small.tile([5, CHID], F32, tag="rce2t")
        nc.vector.tensor_scalar_max(rce2t_sb[:], ce2t_ps[:], 0.0)
        # ce2 (c-partition): [128, 5] per ctile ; then ocp0T accum [5, 64]
        ocp0t_ps = ps_c.tile([5, COUT], F32, tag="c3", name="ocp0t_ps", bufs=1)
        rce2c_sb = [small.tile([128, 5], F32, tag=f"rce2c{t}", name=f"rce2c{t}") for t in range(2)]
        for t in range(2):
            c_ps = ps_c.tile([128, 5], F32, tag="c", name="c_ps")
            nc.tensor.matmul(c_ps[:], gl2t_sb[:, t * 128:(t + 1) * 128], cewt_sb[:],
                             start=True, stop=True)
            nc.vector.tensor_scalar_max(rce2c_sb[t][:], c_ps[:], 0.0)
        for t in range(2):
            nc.tensor.matmul(ocp0t_ps[:], rce2c_sb[t][:], bd2_sb[t][:],
                             start=(t == 0), stop=(t == 1))
        rocp2_sb = small.tile([5, COUT], F32, tag="rocp2")
        nc.vector.tensor_scalar_max(rocp2_sb[:], ocp0t_ps[:], 0.0)
        # gr = rocp2 (bcast over k) * gd2x : [5, 576]
        gr_sb = small.tile([5, 9 * COUT], F32, tag="gr")
        rocp_b = bass.AP(tensor=rocp2_sb[:].tensor, offset=rocp2_sb[:].offset,
                         ap=[list(rocp2_sb[:].ap[0]), [0, 9], [1, COUT]])
        nc.vector.tensor_mul(gr_sb[:], rocp_b, gd2x_sb[:])
        # ocprow [1, 576] = ones5.T @ gr (N=576 -> split 512+64)
        ocprow_ps = ps_c.tile([1, 9 * COUT], F32, tag="c2", name="ocprow_ps", bufs=1)
        nc.tensor.matmul(ocprow_ps[:, 0:512], on5_sb[:], gr_sb[:, 0:512],
                         start=True, stop=True)
        nc.tensor.matmul(ocprow_ps[:, 512:576], on5_sb[:], gr_sb[:, 512:576],
                         start=True, stop=True)
        ocprow_sb = small.tile([1, 9 * COUT], F32, tag="ocprow_sb")
        nc.vector.tensor_copy(ocprow_sb[:], ocprow_ps[:])
        # outTT [128, 9] per ctile
        outtt_sb = [small.tile([128, 9], F32, tag=f"outtt{t}", name=f"outtt{t}") for t in range(2)]
        for t in range(2):
            o_ps = ps_c.tile([128, 9], F32, tag="c", name="o_ps")
            nc.tensor.matmul(o_ps[:], rce2t_sb[:, t * 128:(t + 1) * 128], gdt_sb[:],
                             start=True, stop=True)
            nc.vector.tensor_copy(outtt_sb[t][:], o_ps[:])
        # S = bcast(ocprow) + bcast(outTT); sigmoid; * w2t -> dyn2 [128, 576] x2
        dyn2_sb = [small.tile([128, 9 * COUT], F32R, tag=f"dyn2_{t}", name=f"dyn2_{t}")
                   for t in range(2)]
        for t in range(2):
            bc_ps = ps_c.tile([128, 9 * COUT], F32, tag="c4", name="bc_ps", bufs=1)
            nc.tensor.matmul(bc_ps[:, 0:512], on128_sb[:], ocprow_sb[:, 0:512],
                             start=True, stop=True)
            nc.tensor.matmul(bc_ps[:, 512:576], on128_sb[:], ocprow_sb[:, 512:576],
                             start=True, stop=True)
            s_sb = small.tile([128, 9 * COUT], F32, tag="s_sb")
            ott = outtt_sb[t][:]
            ott_b = bass.AP(tensor=ott.tensor, offset=ott.offset,
                            ap=[list(ott.ap[0]), [1, 9], [0, COUT]])
            nc.vector.tensor_add(s_sb[:], bc_ps[:], ott_b)
            sg_sb = small.tile([128, 9 * COUT], F32, tag="sg2")
            nc.scalar.activation(sg_sb[:], s_sb[:],
                                 mybir.ActivationFunctionType.Sigmoid)
            nc.vector.tensor_mul(dyn2_sb[t][:], sg_sb[:], w2t_sb[t][:])

        # ---- layer 2: 3x3 dynamic conv over own half (dynamic row offset) ----
        ps_c.release()
        ps_y = tc.alloc_tile_pool(name="ps_y", bufs=4, space="PSUM")
        pid = nc.partition_id(engines=[mybir.EngineType.PE])
        off = nc.snap((pid % 2) * (HALF_ROWS * PW), min_val=0,
                      max_val=HALF_ROWS * PW)
        y_sb = big.tile([COUT, HALF], F32, tag="ysb")
        for t0, R in L2_TILES:
            n = PW * R
            yp = ps_y.tile([COUT, n], F32, tag="yp")
            k = 0
            for di in range(3):
                for dj in range(3):
                    base = HB + (t0 + di) * PW + dj - 1
                    for t in range(2):
                        nc.tensor.matmul(
                            yp[:],
                            dyn2_sb[t][:, (3 * di + dj) * COUT:
                                       (3 * di + dj + 1) * COUT],
                            hpad[t][:, bass.ds(off + base, n)],
                            start=(k == 0), stop=(k == 17))
                        k += 1
            src = bass.AP(tensor=yp[:].tensor, offset=yp[:].offset + 1,
                          ap=[list(yp[:].ap[0]), [PW, R], [1, W]])
            nc.vector.tensor_copy(y_sb[:, t0 * W:(t0 + R) * W], src)
            nc.sync.dma_start(y[:, t0 * W:(t0 + R) * W],
                              y_sb[:, t0 * W:(t0 + R) * W])
        ps_y.release()

    nc.finalize()
    return nc


_CACHE = {}


def _get_nc():
    if "nc" not in _CACHE:
        _CACHE["nc"] = _build()
    return _CACHE["nc"]


def _host_weights(fc1_weight, fc1_ce, fc1_gd, fc1_gd2, fc1_ci,
                  fc2_weight, fc2_ce, fc2_gd, fc2_gd2, fc2_ci):
    f = np.float32
    w1 = fc1_weight.reshape(CHID, CIN).astype(f)
    # bd1[c, p*32+o] = fc1_ci[o, c%8] where p = c//8
    bd1 = np.zeros((CIN, CHID), f)
    for c in range(CIN):
        p, g = c // 8, c % 8
        bd1[c, p * 32:(p + 1) * 32] = fc1_ci[:, g]
    # bd2[c, p*2+o] = fc2_ci[o, c%8] where p = c//8
    bd2 = np.zeros((CHID, COUT), f)
    for c in range(CHID):
        p, g = c // 8, c % 8
        bd2[c, p * 2:p * 2 + 2] = fc2_ci[:, g]
    w2t = np.ascontiguousarray(
        fc2_weight.reshape(COUT, CHID, 9).transpose(1, 2, 0).reshape(CHID, 9 * COUT)
    ).astype(f)
    gd2x = np.ascontiguousarray(
        np.repeat(fc2_gd2.T, COUT, axis=1)).astype(f)     # [5, 9*64]
    return {
        "w1t": np.ascontiguousarray(w1.T).astype(f),
        "bd1": bd1,
        "ce1v": np.full((CIN, 1), fc1_ce[0, 0], f),
        "gd1v": np.full((CIN, 1), fc1_gd[0, 0], f),
        "gd21v": np.full((CIN, 1), fc1_gd2[0, 0], f),
        "ones1_64": np.ones((1, CIN), f),
        "ident": np.eye(128, dtype=f),
        "w2t": w2t,
        "bd2": bd2,
        "cewt": np.ascontiguousarray(fc2_ce.T).astype(f),
        "gdt": np.ascontiguousarray(fc2_gd.T).astype(f),
        "gd2x": gd2x,
        "ones5": np.ones((5, 1), f),
        "ones1_128": np.ones((1, 128), f),
    }


def run(inputs, trace=False):
    nc = _get_nc()
    shared = _host_weights(
        inputs["fc1_weight"], inputs["fc1_ce"], inputs["fc1_gd"],
        inputs["fc1_gd2"], inputs["fc1_ci"], inputs["fc2_weight"],
        inputs["fc2_ce"], inputs["fc2_gd"], inputs["fc2_gd2"], inputs["fc2_ci"])
    x = np.asarray(inputs["x"], np.float32)
    in_maps = []
    for core in range(8):
        bi = core // 2
        xb = np.ascontiguousarray(x[bi].reshape(CIN, S))
        in_maps.append({"x64": xb, "x128": xb, **shared})
    res = run_bass_kernel_spmd(nc, in_maps, list(range(8)), trace=trace)
    out = np.empty((B, COUT, H, W), np.float32)
    for core in range(8):
        bi, half = core // 2, core % 2
        out[bi, :, half * HALF_ROWS:(half + 1) * HALF_ROWS, :] = (
            res.results[core]["y"].reshape(COUT, HALF_ROWS, W))
    return out, res


def kernel(**inputs):
    out, _ = run(inputs, trace=False)
    return out


# revision 13
# speedup vs baseline: 1.4479x; 1.0002x over previous
"""Trainium2 Bass kernel for nn_Mlp_70798240907434 (content-gated conv MLP).

Sharding: 8 cores = 4 batches x 2 spatial halves (rows 0-47 / 48-95).
Each core computes the full layer-1 (1x1 dynamic conv + gelu) for its batch
(needed for the global max-pools feeding the dynamic-kernel generation), then
its half of the 3x3 dynamic conv (layer 2). The half offset enters only
through dynamic (register) rhs offsets derived from partition_id, so all 8
cores share one SPMD program. No collectives.

Self-contained: hardcodes shapes from the problem spec.
"""

import contextlib

import numpy as np

import concourse.bass as bass
import concourse.mybir as mybir
import concourse.tile as tile
from concourse import bacc
from concourse.bass_utils import run_bass_kernel_spmd

F32 = mybir.dt.float32
F32R = mybir.dt.float32r

B, CIN, CHID, COUT, H, W = 4, 64, 256, 64, 96, 96
S = H * W                      # 9216
HALF_ROWS = H // 2             # 48
HALF = HALF_ROWS * W           # 4608

# padded h layout: (1+96+1) rows x (1+96+1) cols, flat, +1 front spare +3 back
PW = W + 2                     # 98
HB = 1                         # front spare (tap base can be -1)
HPF = HB + PW * PW + 3         # 9608

# layer-1 spatial tiling: 16-row supertiles, 3 x 512-col matmuls into a
# 3-bank psum tile; 6 tiles (1:1 with the x chunks)
L1_ROWS = 16
L1_NT = H // L1_ROWS           # 6
L1_N = 512                     # cols per matmul
XCHUNK_ROWS = 16               # x loaded in 6 chunks of 16 rows
NXCH = H // XCHUNK_ROWS        # 6

# layer-2 spatial tiling (own half): 5-row tiles in padded coords
L2_ROWS = 5
L2_TILES = [(t0, min(L2_ROWS, HALF_ROWS - t0)) for t0 in range(0, HALF_ROWS, L2_ROWS)]


def _build():
    nc = bacc.Bacc()

    # ---- DRAM parameters (per-core) ----
    x64 = nc.declare_dram_parameter("x64", [CIN, S], F32R, isOutput=False)
    x128 = nc.declare_dram_parameter("x128", [CIN, S], F32R, isOutput=False)
    w1t = nc.declare_dram_parameter("w1t", [CIN, CHID], F32, isOutput=False)
    bd1 = nc.declare_dram_parameter("bd1", [CIN, CHID], F32, isOutput=False)
    ce1v = nc.declare_dram_parameter("ce1v", [CIN, 1], F32, isOutput=False)
    gd1v = nc.declare_dram_parameter("gd1v", [CIN, 1], F32, isOutput=False)
    gd21v = nc.declare_dram_parameter("gd21v", [CIN, 1], F32, isOutput=False)
    ones1_64 = nc.declare_dram_parameter("ones1_64", [1, CIN], F32, isOutput=False)
    ident = nc.declare_dram_parameter("ident", [128, 128], F32, isOutput=False)
    w2t = nc.declare_dram_parameter("w2t", [CHID, 9 * COUT], F32, isOutput=False)
    bd2 = nc.declare_dram_parameter("bd2", [CHID, COUT], F32, isOutput=False)
    cewt = nc.declare_dram_parameter("cewt", [9, 5], F32, isOutput=False)
    gdt = nc.declare_dram_parameter("gdt", [5, 9], F32, isOutput=False)
    gd2x = nc.declare_dram_parameter("gd2x", [5, 9 * COUT], F32, isOutput=False)
    ones5 = nc.declare_dram_parameter("ones5", [5, 1], F32, isOutput=False)
    ones1_128 = nc.declare_dram_parameter("ones1_128", [1, 128], F32, isOutput=False)
    y = nc.declare_dram_parameter("y", [COUT, HALF], F32, isOutput=True)

    with tile.TileContext(nc) as tc, contextlib.ExitStack() as ctx:
        consts = ctx.enter_context(tc.tile_pool(name="consts", bufs=1))
        big = ctx.enter_context(tc.tile_pool(name="big", bufs=1))
        small = ctx.enter_context(tc.tile_pool(name="small", bufs=2))

        # ---- load small constants ----
        w1t_sb = consts.tile([CIN, CHID], F32, tag="w1t")
        bd1_sb = consts.tile([CIN, CHID], F32, tag="bd1")
        ce1_sb = consts.tile([CIN, 1], F32, tag="ce1")
        gd1_sb = consts.tile([CIN, 1], F32, tag="gd1")
        gd21_sb = consts.tile([CIN, 1], F32, tag="gd21")
        on64_sb = consts.tile([1, CIN], F32, tag="on64")
        id_sb = consts.tile([128, 128], F32, tag="ident")
        w2t_sb = [consts.tile([128, 9 * COUT], F32, tag=f"w2t{t}", name=f"w2t{t}") for t in range(2)]
        bd2_sb = [consts.tile([128, COUT], F32, tag=f"bd2{t}", name=f"bd2{t}") for t in range(2)]
        cewt_sb = consts.tile([9, 5], F32, tag="cewt")
        gdt_sb = consts.tile([5, 9], F32, tag="gdt")
        gd2x_sb = consts.tile([5, 9 * COUT], F32, tag="gd2x")
        on5_sb = consts.tile([5, 1], F32, tag="on5")
        on128_sb = consts.tile([1, 128], F32, tag="on128")
        for t, d in [
            (w1t_sb, w1t), (bd1_sb, bd1), (ce1_sb, ce1v), (gd1_sb, gd1v),
            (gd21_sb, gd21v), (on64_sb, ones1_64), (id_sb, ident),
            (cewt_sb, cewt), (gdt_sb, gdt), (gd2x_sb, gd2x),
            (on5_sb, ones5), (on128_sb, ones1_128),
        ]:
            nc.scalar.dma_start(t[:], d[:])
        for t in range(2):
            nc.scalar.dma_start(w2t_sb[t][:], w2t[t * 128:(t + 1) * 128, :])
            nc.scalar.dma_start(bd2_sb[t][:], bd2[t * 128:(t + 1) * 128, :])

        # ---- x loads ----
        # x64: [64, 9216] (c partitions) in 8 row-chunks for the L1 matmuls
        xch = [consts.tile([CIN, XCHUNK_ROWS * W], F32R, tag=f"xch{k}", name=f"xch{k}")
               for k in range(NXCH)]
        for k in range(NXCH):
            nc.sync.dma_start(
                xch[k][:], x64[:, k * XCHUNK_ROWS * W:(k + 1) * XCHUNK_ROWS * W])

        # ---- h_pad tiles (padded gelu output), zero the pad regions ----
        hpad = [big.tile([128, HPF], F32R, tag=f"hpad{t}", name=f"hpad{t}") for t in range(2)]
        for t in range(2):
            hp = hpad[t][:].bitcast(F32)
            # front spare + top pad row
            nc.vector.memset(hp[:, 0:HB + PW], 0.0)
            # bottom pad row + back spare
            nc.vector.memset(hp[:, HB + 97 * PW:HPF], 0.0)
            # left/right pad cols of rows 1..96: offset HB+PW, [(PW,96),(97,2)]
            colpad = bass.AP(
                tensor=hp.tensor, offset=HB + PW,
                ap=[list(hp.ap[0]), [PW, 96], [97, 2]])
            nc.vector.memset(colpad, 0.0)

        # ---- gl1: global per-channel max of x (from the x64 chunks) ----
        xmaxc = small.tile([CIN, NXCH], F32, tag="xmaxc")
        for k in range(NXCH):
            nc.vector.reduce_max(xmaxc[:, k:k + 1], xch[k][:],
                                 axis=mybir.AxisListType.X)
        gl1_sb = small.tile([CIN, 1], F32, tag="gl1")
        nc.vector.reduce_max(gl1_sb[:], xmaxc[:], axis=mybir.AxisListType.X)

        # ---- dyn1 generation ----
        rce1_sb = small.tile([CIN, 1], F32, tag="rce1")
        nc.vector.tensor_scalar(rce1_sb[:], gl1_sb[:], ce1_sb[:], 0.0,
                                mybir.AluOpType.mult, mybir.AluOpType.max)
        outc_sb = small.tile([CIN, 1], F32, tag="outc")
        nc.vector.tensor_scalar_mul(outc_sb[:], rce1_sb[:], gd1_sb[:])
        ps_a = tc.alloc_tile_pool(name="ps_a", bufs=2, space="PSUM")
        ocp0_ps = ps_a.tile([1, CHID], F32, tag="a", name="ocp0_ps")
        nc.tensor.matmul(ocp0_ps[:], rce1_sb[:], bd1_sb[:], start=True, stop=True)
        rocp1_sb = small.tile([1, CHID], F32, tag="rocp1")
        nc.vector.tensor_scalar_max(rocp1_sb[:], ocp0_ps[:], 0.0)
        sig1_ps = ps_a.tile([CIN, CHID], F32, tag="a", name="sig1_ps")
        nc.tensor.matmul(sig1_ps[:], on64_sb[:], rocp1_sb[:], start=True, stop=True)
        sg1_sb = small.tile([CIN, CHID], F32, tag="sg1")
        nc.scalar.activation(sg1_sb[:], sig1_ps[:],
                             mybir.ActivationFunctionType.Sigmoid,
                             bias=outc_sb[:], scale=gd21_sb[:])
        dyn1_sb = small.tile([CIN, CHID], F32R, tag="dyn1")
        nc.vector.tensor_mul(dyn1_sb[:], sg1_sb[:], w1t_sb[:])

        # ---- layer 1: z = dyn1.T @ x ; h = gelu(z) -> hpad; pool stage A ----
        ps_a.release()
        ps_big = tc.alloc_tile_pool(name="ps_big", bufs=2, space="PSUM")
        stageA = [big.tile([128, H * 3], F32, tag=f"stA{t}", name=f"stA{t}") for t in range(2)]
        for j in range(L1_NT):          # 16-row supertiles, 1:1 with x chunks
            for m in range(2):          # oc tile
                z_ps = ps_big.tile([128, 3, 512], F32, tag="z")
                for i in range(3):      # three 512-col matmuls, one bank each
                    rhs = xch[j][:, i * L1_N:(i + 1) * L1_N]
                    nc.tensor.matmul(z_ps[:, i, :],
                                     dyn1_sb[:, m * 128:(m + 1) * 128], rhs,
                                     start=True, stop=True)
                # gelu eviction into padded layout (rows 16j..16j+15); the
                # 3x512 psum banks are contiguous per partition = 16 rows
                dst = bass.AP(
                    tensor=hpad[m][:].tensor,
                    offset=HB + (16 * j + 1) * PW + 1,
                    ap=[list(hpad[m][:].ap[0]), [PW, L1_ROWS], [1, W]])
                src = bass.AP(tensor=z_ps[:].tensor, offset=z_ps[:].offset,
                              ap=[list(z_ps[:].ap[0]), [W, L1_ROWS], [1, W]])
                nc.scalar.activation(dst, src, mybir.ActivationFunctionType.Gelu)
                # pool stage A from post-gelu h: per-row 32-col maxes
                # (gelu is NOT monotone, so the pool must read h, not z)
                hpf32 = hpad[m][:].bitcast(F32)
                pin = bass.AP(tensor=hpf32.tensor,
                              offset=HB + (16 * j + 1) * PW + 1,
                              ap=[list(hpf32.ap[0]), [PW, L1_ROWS], [32, 3], [1, 32]])
                nc.vector.reduce_max(
                    stageA[m][:, 16 * j * 3:(16 * j + L1_ROWS) * 3], pin,
                    axis=mybir.AxisListType.X)

        # ---- pool stage B -> gl2 [128, 9] per ctile ----
        gl2_sb = [small.tile([128, 9], F32, tag=f"gl2_{t}", name=f"gl2_{t}") for t in range(2)]
        for t in range(2):
            sA = stageA[t][:]
            pin = bass.AP(tensor=sA.tensor, offset=sA.offset,
                          ap=[list(sA.ap[0]), [96, 3], [1, 3], [3, 32]])
            nc.vector.reduce_max(gl2_sb[t][:], pin, axis=mybir.AxisListType.X)

        # ---- dyn2 generation ----
        ps_big.release()
        ps_c = tc.alloc_tile_pool(name="ps_c", bufs=2, space="PSUM")
        # gl2T [9, 256]
        gl2t_sb = small.tile([9, CHID], F32, tag="gl2t")
        for t in range(2):
            tp_ps = ps_c.tile([9, 128], F32, tag="c", name="tp_ps")
            nc.tensor.transpose(tp_ps[:], gl2_sb[t][:], id_sb[:])
            nc.vector.tensor_copy(gl2t_sb[:, t * 128:(t + 1) * 128], tp_ps[:])
        # ce2T = cewt.T @ gl2T : [5, 256]
        ce2t_ps = ps_c.tile([5, CHID], F32, tag="c2", name="ce2t_ps", bufs=1)
        nc.tensor.matmul(ce2t_ps[:], cewt_sb[:], gl2t_sb[:], start=True, stop=True)
        rce2t_sb = small.tile([5, CHID], F32, tag="rce2t")
        nc.vector.tensor_scalar_max(rce2t_sb[:], ce2t_ps[:], 0.0)
        # ce2 (c-partition): [128, 5] per ctile ; then ocp0T accum [5, 64]
        ocp0t_ps = ps_c.tile([5, COUT], F32, tag="c3", name="ocp0t_ps", bufs=1)
        rce2c_sb = [small.tile([128, 5], F32, tag=f"rce2c{t}", name=f"rce2c{t}") for t in range(2)]
        for t in range(2):
            c_ps = ps_c.tile([128, 5], F32, tag="c", name="c_ps")
            nc.tensor.matmul(c_ps[:], gl2t_sb[:, t * 128:(t + 1) * 128], cewt_sb[:],
                             start=True, stop=True)
            nc.vector.tensor_scalar_max(rce2c_sb[t][:], c_ps[:], 0.0)
        for t in range(2):
            nc.tensor.matmul(ocp0t_ps[:], rce2c_sb[t][:], bd2_sb[t][:],
                             start=(t == 0), stop=(t == 1))
        rocp2_sb = small.tile([5, COUT], F32, tag="rocp2")
        nc.vector.tensor_scalar_max(rocp2_sb[:], ocp0t_ps[:], 0.0)
        # gr = rocp2 (bcast over k) * gd2x : [5, 576]
        gr_sb = small.tile([5, 9 * COUT], F32, tag="gr")
        rocp_b = bass.AP(tensor=rocp2_sb[:].tensor, offset=rocp2_sb[:].offset,
                         ap=[list(rocp2_sb[:].ap[0]), [0, 9], [1, COUT]])
        nc.vector.tensor_mul(gr_sb[:], rocp_b, gd2x_sb[:])
        # ocprow [1, 576] = ones5.T @ gr (N=576 -> split 512+64)
        ocprow_ps = ps_c.tile([1, 9 * COUT], F32, tag="c2", name="ocprow_ps", bufs=1)
        nc.tensor.matmul(ocprow_ps[:, 0:512], on5_sb[:], gr_sb[:, 0:512],
                         start=True, stop=True)
        nc.tensor.matmul(ocprow_ps[:, 512:576], on5_sb[:], gr_sb[:, 512:576],
                         start=True, stop=True)
        ocprow_sb = small.tile([1, 9 * COUT], F32, tag="ocprow_sb")
        nc.vector.tensor_copy(ocprow_sb[:], ocprow_ps[:])
        # outTT [128, 9] per ctile
        outtt_sb = [small.tile([128, 9], F32, tag=f"outtt{t}", name=f"outtt{t}") for t in range(2)]
        for t in range(2):
            o_ps = ps_c.tile([128, 9], F32, tag="c", name="o_ps")
            nc.tensor.matmul(o_ps[:], rce2t_sb[:, t * 128:(t + 1) * 128], gdt_sb[:],
                             start=True, stop=True)
            nc.vector.tensor_copy(outtt_sb[t][:], o_ps[:])
        # S = bcast(ocprow) + bcast(outTT); sigmoid; * w2t -> dyn2 [128, 576] x2
        dyn2_sb = [small.tile([128, 9 * COUT], F32R, tag=f"dyn2_{t}", name=f"dyn2_{t}")
                   for t in range(2)]
        for t in range(2):
            bc_ps = ps_c.tile([128, 9 * COUT], F32, tag="c4", name="bc_ps", bufs=1)
            nc.tensor.matmul(bc_ps[:, 0:512], on128_sb[:], ocprow_sb[:, 0:512],
                             start=True, stop=True)
            nc.tensor.matmul(bc_ps[:, 512:576], on128_sb[:], ocprow_sb[:, 512:576],
                             start=True, stop=True)
            s_sb = small.tile([128, 9 * COUT], F32, tag="s_sb")
            ott = outtt_sb[t][:]
            ott_b = bass.AP(tensor=ott.tensor, offset=ott.offset,
                            ap=[list(ott.ap[0]), [1, 9], [0, COUT]])
            nc.vector.tensor_add(s_sb[:], bc_ps[:], ott_b)
            sg_sb = small.tile([128, 9 * COUT], F32, tag="sg2")
            nc.scalar.activation(sg_sb[:], s_sb[:],
                                 mybir.ActivationFunctionType.Sigmoid)
            nc.vector.tensor_mul(dyn2_sb[t][:], sg_sb[:], w2t_sb[t][:])

        # ---- layer 2: 3x3 dynamic conv over own half (dynamic row offset) ----
        ps_c.release()
        ps_y = tc.alloc_tile_pool(name="ps_y", bufs=6, space="PSUM")
        pid = nc.partition_id(engines=[mybir.EngineType.PE])
        off = nc.snap((pid % 2) * (HALF_ROWS * PW), min_val=0,
                      max_val=HALF_ROWS * PW)
        y_sb = big.tile([COUT, HALF], F32, tag="ysb")
        for t0, R in L2_TILES:
            n = PW * R
            yp = ps_y.tile([COUT, n], F32, tag="yp")
            k = 0
            for di in range(3):
                for dj in range(3):
                    base = HB + (t0 + di) * PW + dj - 1
                    for t in range(2):
                        nc.tensor.matmul(
                            yp[:],
                            dyn2_sb[t][:, (3 * di + dj) * COUT:
                                       (3 * di + dj + 1) * COUT],
                            hpad[t][:, bass.ds(off + base, n)],
                            start=(k == 0), stop=(k == 17))
                        k += 1
            src = bass.AP(tensor=yp[:].tensor, offset=yp[:].offset + 1,
                          ap=[list(yp[:].ap[0]), [PW, R], [1, W]])
            nc.vector.tensor_copy(y_sb[:, t0 * W:(t0 + R) * W], src)
            nc.sync.dma_start(y[:, t0 * W:(t0 + R) * W],
                              y_sb[:, t0 * W:(t0 + R) * W])
        ps_y.release()

    nc.finalize()
    return nc


_CACHE = {}


def _get_nc():
    if "nc" not in _CACHE:
        _CACHE["nc"] = _build()
    return _CACHE["nc"]


def _host_weights(fc1_weight, fc1_ce, fc1_gd, fc1_gd2, fc1_ci,
                  fc2_weight, fc2_ce, fc2_gd, fc2_gd2, fc2_ci):
    f = np.float32
    w1 = fc1_weight.reshape(CHID, CIN).astype(f)
    # bd1[c, p*32+o] = fc1_ci[o, c%8] where p = c//8
    bd1 = np.zeros((CIN, CHID), f)
    for c in range(CIN):
        p, g = c // 8, c % 8
        bd1[c, p * 32:(p + 1) * 32] = fc1_ci[:, g]
    # bd2[c, p*2+o] = fc2_ci[o, c%8] where p = c//8
    bd2 = np.zeros((CHID, COUT), f)
    for c in range(CHID):
        p, g = c // 8, c % 8
        bd2[c, p * 2:p * 2 + 2] = fc2_ci[:, g]
    w2t = np.ascontiguousarray(
        fc2_weight.reshape(COUT, CHID, 9).transpose(1, 2, 0).reshape(CHID, 9 * COUT)
    ).astype(f)
    gd2x = np.ascontiguousarray(
        np.repeat(fc2_gd2.T, COUT, axis=1)).astype(f)     # [5, 9*64]
    return {
        "w1t": np.ascontiguousarray(w1.T).astype(f),
        "bd1": bd1,
        "ce1v": np.full((CIN, 1), fc1_ce[0, 0], f),
        "gd1v": np.full((CIN, 1), fc1_gd[0, 0], f),
        "gd21v": np.full((CIN, 1), fc1_gd2[0, 0], f),
        "ones1_64": np.ones((1, CIN), f),
        "ident": np.eye(128, dtype=f),
        "w2t": w2t,
        "bd2": bd2,
        "cewt": np.ascontiguousarray(fc2_ce.T).astype(f),
        "gdt": np.ascontiguousarray(fc2_gd.T).astype(f),
        "gd2x": gd2x,
        "ones5": np.ones((5, 1), f),
        "ones1_128": np.ones((1, 128), f),
    }


def run(inputs, trace=False):
    nc = _get_nc()
    shared = _host_weights(
        inputs["fc1_weight"], inputs["fc1_ce"], inputs["fc1_gd"],
        inputs["fc1_gd2"], inputs["fc1_ci"], inputs["fc2_weight"],
        inputs["fc2_ce"], inputs["fc2_gd"], inputs["fc2_gd2"], inputs["fc2_ci"])
    x = np.asarray(inputs["x"], np.float32)
    in_maps = []
    for core in range(8):
        bi = core // 2
        xb = np.ascontiguousarray(x[bi].reshape(CIN, S))
        in_maps.append({"x64": xb, "x128": xb, **shared})
    res = run_bass_kernel_spmd(nc, in_maps, list(range(8)), trace=trace)
    out = np.empty((B, COUT, H, W), np.float32)
    for core in range(8):
        bi, half = core // 2, core % 2
        out[bi, :, half * HALF_ROWS:(half + 1) * HALF_ROWS, :] = (
            res.results[core]["y"].reshape(COUT, HALF_ROWS, W))
    return out, res


def kernel(**inputs):
    out, _ = run(inputs, trace=False)
    return out


# revision 16
# speedup vs baseline: 1.4526x; 1.0032x over previous
"""Trainium2 Bass kernel for nn_Mlp_70798240907434 (content-gated conv MLP).

Sharding: 8 cores = 4 batches x 2 spatial halves (rows 0-47 / 48-95).
Each core computes the full layer-1 (1x1 dynamic conv + gelu) for its batch
(needed for the global max-pools feeding the dynamic-kernel generation), then
its half of the 3x3 dynamic conv (layer 2). The half offset enters only
through dynamic (register) rhs offsets derived from partition_id, so all 8
cores share one SPMD program. No collectives.

Self-contained: hardcodes shapes from the problem spec.
"""

import contextlib

import numpy as np

import concourse.bass as bass
import concourse.mybir as mybir
import concourse.tile as tile
from concourse import bacc
from concourse.bass_utils import run_bass_kernel_spmd

F32 = mybir.dt.float32
F32R = mybir.dt.float32r

B, CIN, CHID, COUT, H, W = 4, 64, 256, 64, 96, 96
S = H * W                      # 9216
HALF_ROWS = H // 2             # 48
HALF = HALF_ROWS * W           # 4608

# padded h layout: (1+96+1) rows x (1+96+1) cols, flat, +1 front spare +3 back
PW = W + 2                     # 98
HB = 1                         # front spare (tap base can be -1)
HPF = HB + PW * PW + 3         # 9608

# layer-1 spatial tiling: 16-row supertiles, 3 x 512-col matmuls into a
# 3-bank psum tile; 6 tiles (1:1 with the x chunks)
L1_ROWS = 16
L1_NT = H // L1_ROWS           # 6
L1_N = 512                     # cols per matmul
XCHUNK_ROWS = 16               # x loaded in 6 chunks of 16 rows
NXCH = H // XCHUNK_ROWS        # 6

# layer-2 spatial tiling (own half): 5-row tiles in padded coords
L2_ROWS = 5
L2_TILES = [(t0, min(L2_ROWS, HALF_ROWS - t0)) for t0 in range(0, HALF_ROWS, L2_ROWS)]


def _build():
    nc = bacc.Bacc()

    # ---- DRAM parameters (per-core) ----
    x64 = nc.declare_dram_parameter("x64", [CIN, S], F32R, isOutput=False)
    x128 = nc.declare_dram_parameter("x128", [CIN, S], F32R, isOutput=False)
    w1t = nc.declare_dram_parameter("w1t", [CIN, CHID], F32, isOutput=False)
    bd1 = nc.declare_dram_parameter("bd1", [CIN, CHID], F32, isOutput=False)
    ce1v = nc.declare_dram_parameter("ce1v", [CIN, 1], F32, isOutput=False)
    gd1v = nc.declare_dram_parameter("gd1v", [CIN, 1], F32, isOutput=False)
    gd21v = nc.declare_dram_parameter("gd21v", [CIN, 1], F32, isOutput=False)
    ones1_64 = nc.declare_dram_parameter("ones1_64", [1, CIN], F32, isOutput=False)
    ident = nc.declare_dram_parameter("ident", [128, 128], F32, isOutput=False)
    w2t = nc.declare_dram_parameter("w2t", [CHID, 9 * COUT], F32, isOutput=False)
    bd2 = nc.declare_dram_parameter("bd2", [CHID, COUT], F32, isOutput=False)
    cewt = nc.declare_dram_parameter("cewt", [9, 5], F32, isOutput=False)
    gdt = nc.declare_dram_parameter("gdt", [5, 9], F32, isOutput=False)
    gd2x = nc.declare_dram_parameter("gd2x", [5, 9 * COUT], F32, isOutput=False)
    ones5 = nc.declare_dram_parameter("ones5", [5, 1], F32, isOutput=False)
    ones1_128 = nc.declare_dram_parameter("ones1_128", [1, 128], F32, isOutput=False)
    y = nc.declare_dram_parameter("y", [COUT, HALF], F32, isOutput=True)

    with tile.TileContext(nc) as tc, contextlib.ExitStack() as ctx:
        consts = ctx.enter_context(tc.tile_pool(name="consts", bufs=1))
        big = ctx.enter_context(tc.tile_pool(name="big", bufs=1))
        small = ctx.enter_context(tc.tile_pool(name="small", bufs=2))

        # ---- load small constants ----
        w1t_sb = consts.tile([CIN, CHID], F32, tag="w1t")
        bd1_sb = consts.tile([CIN, CHID], F32, tag="bd1")
        ce1_sb = consts.tile([CIN, 1], F32, tag="ce1")
        gd1_sb = consts.tile([CIN, 1], F32, tag="gd1")
        gd21_sb = consts.tile([CIN, 1], F32, tag="gd21")
        on64_sb = consts.tile([1, CIN], F32, tag="on64")
        id_sb = consts.tile([128, 128], F32, tag="ident")
        w2t_sb = [consts.tile([128, 9 * COUT], F32, tag=f"w2t{t}", name=f"w2t{t}") for t in range(2)]
        bd2_sb = [consts.tile([128, COUT], F32, tag=f"bd2{t}", name=f"bd2{t}") for t in range(2)]
        cewt_sb = consts.tile([9, 5], F32, tag="cewt")
        gdt_sb = consts.tile([5, 9], F32, tag="gdt")
        gd2x_sb = consts.tile([5, 9 * COUT], F32, tag="gd2x")
        on5_sb = consts.tile([5, 1], F32, tag="on5")
        on128_sb = consts.tile([1, 128], F32, tag="on128")
        for t, d in [
            (w1t_sb, w1t), (bd1_sb, bd1), (ce1_sb, ce1v), (gd1_sb, gd1v),
            (gd21_sb, gd21v), (on64_sb, ones1_64), (id_sb, ident),
            (cewt_sb, cewt), (gdt_sb, gdt), (gd2x_sb, gd2x),
            (on5_sb, ones5), (on128_sb, ones1_128),
        ]:
            nc.scalar.dma_start(t[:], d[:])
        for t in range(2):
            nc.scalar.dma_start(w2t_sb[t][:], w2t[t * 128:(t + 1) * 128, :])
            nc.scalar.dma_start(bd2_sb[t][:], bd2[t * 128:(t + 1) * 128, :])

        # ---- x loads ----
        # x64: [64, 9216] (c partitions) in 8 row-chunks for the L1 matmuls
        xch = [consts.tile([CIN, XCHUNK_ROWS * W], F32R, tag=f"xch{k}", name=f"xch{k}")
               for k in range(NXCH)]
        for k in range(NXCH):
            nc.sync.dma_start(
                xch[k][:], x64[:, k * XCHUNK_ROWS * W:(k + 1) * XCHUNK_ROWS * W])

        # ---- h_pad tiles (padded gelu output), zero the pad regions ----
        hpad = [big.tile([128, HPF], F32R, tag=f"hpad{t}", name=f"hpad{t}") for t in range(2)]
        for t in range(2):
            hp = hpad[t][:].bitcast(F32)
            # front spare + top pad row
            nc.vector.memset(hp[:, 0:HB + PW], 0.0)
            # bottom pad row + back spare
            nc.vector.memset(hp[:, HB + 97 * PW:HPF], 0.0)
            # left/right pad cols of rows 1..96: offset HB+PW, [(PW,96),(97,2)]
            colpad = bass.AP(
                tensor=hp.tensor, offset=HB + PW,
                ap=[list(hp.ap[0]), [PW, 96], [97, 2]])
            nc.vector.memset(colpad, 0.0)

        # ---- gl1: global per-channel max of x (from the x64 chunks) ----
        xmaxc = small.tile([CIN, NXCH], F32, tag="xmaxc")
        for k in range(NXCH):
            nc.vector.reduce_max(xmaxc[:, k:k + 1], xch[k][:],
                                 axis=mybir.AxisListType.X)
        gl1_sb = small.tile([CIN, 1], F32, tag="gl1")
        nc.vector.reduce_max(gl1_sb[:], xmaxc[:], axis=mybir.AxisListType.X)

        # ---- dyn1 generation ----
        rce1_sb = small.tile([CIN, 1], F32, tag="rce1")
        nc.vector.tensor_scalar(rce1_sb[:], gl1_sb[:], ce1_sb[:], 0.0,
                                mybir.AluOpType.mult, mybir.AluOpType.max)
        outc_sb = small.tile([CIN, 1], F32, tag="outc")
        nc.vector.tensor_scalar_mul(outc_sb[:], rce1_sb[:], gd1_sb[:])
        ps_a = tc.alloc_tile_pool(name="ps_a", bufs=2, space="PSUM")
        ocp0_ps = ps_a.tile([1, CHID], F32, tag="a", name="ocp0_ps")
        nc.tensor.matmul(ocp0_ps[:], rce1_sb[:], bd1_sb[:], start=True, stop=True)
        rocp1_sb = small.tile([1, CHID], F32, tag="rocp1")
        nc.vector.tensor_scalar_max(rocp1_sb[:], ocp0_ps[:], 0.0)
        sig1_ps = ps_a.tile([CIN, CHID], F32, tag="a", name="sig1_ps")
        nc.tensor.matmul(sig1_ps[:], on64_sb[:], rocp1_sb[:], start=True, stop=True)
        sg1_sb = small.tile([CIN, CHID], F32, tag="sg1")
        nc.scalar.activation(sg1_sb[:], sig1_ps[:],
                             mybir.ActivationFunctionType.Sigmoid,
                             bias=outc_sb[:], scale=gd21_sb[:])
        dyn1_sb = small.tile([CIN, CHID], F32R, tag="dyn1")
        nc.vector.tensor_mul(dyn1_sb[:], sg1_sb[:], w1t_sb[:])

        # ---- layer 1: z = dyn1.T @ x ; h = gelu(z) -> hpad; pool stage A ----
        ps_a.release()
        ps_big = tc.alloc_tile_pool(name="ps_big", bufs=2, space="PSUM")
        stageA = [big.tile([128, H * 3], F32, tag=f"stA{t}", name=f"stA{t}") for t in range(2)]
        for j in range(L1_NT):          # 16-row supertiles, 1:1 with x chunks
            for m in range(2):          # oc tile
                z_ps = ps_big.tile([128, 3, 512], F32, tag="z")
                for i in range(3):      # three 512-col matmuls, one bank each
                    rhs = xch[j][:, i * L1_N:(i + 1) * L1_N]
                    nc.tensor.matmul(z_ps[:, i, :],
                                     dyn1_sb[:, m * 128:(m + 1) * 128], rhs,
                                     start=True, stop=True)
                # gelu eviction into padded layout (rows 16j..16j+15); the
                # 3x512 psum banks are contiguous per partition = 16 rows
                dst = bass.AP(
                    tensor=hpad[m][:].tensor,
                    offset=HB + (16 * j + 1) * PW + 1,
                    ap=[list(hpad[m][:].ap[0]), [PW, L1_ROWS], [1, W]])
                src = bass.AP(tensor=z_ps[:].tensor, offset=z_ps[:].offset,
                              ap=[list(z_ps[:].ap[0]), [W, L1_ROWS], [1, W]])
                nc.scalar.activation(dst, src, mybir.ActivationFunctionType.Gelu)
                # pool stage A from post-gelu h: per-row 32-col maxes
                # (gelu is NOT monotone, so the pool must read h, not z)
                hpf32 = hpad[m][:].bitcast(F32)
                pin = bass.AP(tensor=hpf32.tensor,
                              offset=HB + (16 * j + 1) * PW + 1,
                              ap=[list(hpf32.ap[0]), [PW, L1_ROWS], [32, 3], [1, 32]])
                nc.vector.reduce_max(
                    stageA[m][:, 16 * j * 3:(16 * j + L1_ROWS) * 3], pin,
                    axis=mybir.AxisListType.X)

        # ---- pool stage B -> gl2 [128, 9] per ctile ----
        gl2_sb = [small.tile([128, 9], F32, tag=f"gl2_{t}", name=f"gl2_{t}") for t in range(2)]
        for t in range(2):
            sA = stageA[t][:]
            pin = bass.AP(tensor=sA.tensor, offset=sA.offset,
                          ap=[list(sA.ap[0]), [96, 3], [1, 3], [3, 32]])
            nc.vector.reduce_max(gl2_sb[t][:], pin, axis=mybir.AxisListType.X)

        # ---- dyn2 generation ----
        ps_big.release()
        ps_c = tc.alloc_tile_pool(name="ps_c", bufs=2, space="PSUM")
        # gl2T [9, 256]
        gl2t_sb = small.tile([9, CHID], F32, tag="gl2t")
        for t in range(2):
            tp_ps = ps_c.tile([9, 128], F32, tag="c", name="tp_ps")
            nc.tensor.transpose(tp_ps[:], gl2_sb[t][:], id_sb[:])
            nc.vector.tensor_copy(gl2t_sb[:, t * 128:(t + 1) * 128], tp_ps[:])
        # ce2T = cewt.T @ gl2T : [5, 256]
        ce2t_ps = ps_c.tile([5, CHID], F32, tag="c2", name="ce2t_ps", bufs=1)
        nc.tensor.matmul(ce2t_ps[:], cewt_sb[:], gl2t_sb[:], start=True, stop=True)
        rce2t_sb = small.tile([5, CHID], F32, tag="rce2t")
        nc.vector.tensor_scalar_max(rce2t_sb[:], ce2t_ps[:], 0.0)
        # ce2 (c-partition): [128, 5] per ctile ; then ocp0T accum [5, 64]
        ocp0t_ps = ps_c.tile([5, COUT], F32, tag="c3", name="ocp0t_ps", bufs=1)
        rce2c_sb = [small.tile([128, 5], F32, tag=f"rce2c{t}", name=f"rce2c{t}") for t in range(2)]
        for t in range(2):
            c_ps = ps_c.tile([128, 5], F32, tag="c", name="c_ps")
            nc.tensor.matmul(c_ps[:], gl2t_sb[:, t * 128:(t + 1) * 128], cewt_sb[:],
                             start=True, stop=True)
            nc.vector.tensor_scalar_max(rce2c_sb[t][:], c_ps[:], 0.0)
        for t in range(2):
            nc.tensor.matmul(ocp0t_ps[:], rce2c_sb[t][:], bd2_sb[t][:],
                             start=(t == 0), stop=(t == 1))
        rocp2_sb = small.tile([5, COUT], F32, tag="rocp2")
        nc.vector.tensor_scalar_max(rocp2_sb[:], ocp0t_ps[:], 0.0)
        # gr = rocp2 (bcast over k) * gd2x : [5, 576]
        gr_sb = small.tile([5, 9 * COUT], F32, tag="gr")
        rocp_b = bass.AP(tensor=rocp2_sb[:].tensor, offset=rocp2_sb[:].offset,
                         ap=[list(rocp2_sb[:].ap[0]), [0, 9], [1, COUT]])
        nc.vector.tensor_mul(gr_sb[:], rocp_b, gd2x_sb[:])
        # ocprow [1, 576] = ones5.T @ gr (N=576 -> split 512+64)
        ocprow_ps = ps_c.tile([1, 9 * COUT], F32, tag="c2", name="ocprow_ps", bufs=1)
        nc.tensor.matmul(ocprow_ps[:, 0:512], on5_sb[:], gr_sb[:, 0:512],
                         start=True, stop=True)
        nc.tensor.matmul(ocprow_ps[:, 512:576], on5_sb[:], gr_sb[:, 512:576],
                         start=True, stop=True)
        ocprow_sb = small.tile([1, 9 * COUT], F32, tag="ocprow_sb")
        nc.vector.tensor_copy(ocprow_sb[:], ocprow_ps[:])
        # outTT [128, 9] per ctile
        outtt_sb = [small.tile([128, 9], F32, tag=f"outtt{t}", name=f"outtt{t}") for t in range(2)]
        for t in range(2):
            o_ps = ps_c.tile([128, 9], F32, tag="c", name="o_ps")
            nc.tensor.matmul(o_ps[:], rce2t_sb[:, t * 128:(t + 1) * 128], gdt_sb[:],
                             start=True, stop=True)
            nc.vector.tensor_copy(outtt_sb[t][:], o_ps[:])
        # S = bcast(ocprow) + bcast(outTT); sigmoid; * w2t -> dyn2 [128, 576] x2
        dyn2_sb = [small.tile([128, 9 * COUT], F32R, tag=f"dyn2_{t}", name=f"dyn2_{t}")
                   for t in range(2)]
        for t in range(2):
            bc_ps = ps_c.tile([128, 9 * COUT], F32, tag="c4", name="bc_ps", bufs=1)
            nc.tensor.matmul(bc_ps[:, 0:512], on128_sb[:], ocprow_sb[:, 0:512],
                             start=True, stop=True)
            nc.tensor.matmul(bc_ps[:, 512:576], on128_sb[:], ocprow_sb[:, 512:576],
                             start=True, stop=True)
            s_sb = small.tile([128, 9 * COUT], F32, tag="s_sb")
            ott = outtt_sb[t][:]
            ott_b = bass.AP(tensor=ott.tensor, offset=ott.offset,
                            ap=[list(ott.ap[0]), [1, 9], [0, COUT]])
            nc.vector.tensor_add(s_sb[:], bc_ps[:], ott_b)
            sg_sb = small.tile([128, 9 * COUT], F32, tag="sg2")
            nc.scalar.activation(sg_sb[:], s_sb[:],
                                 mybir.ActivationFunctionType.Sigmoid)
            nc.vector.tensor_mul(dyn2_sb[t][:], sg_sb[:], w2t_sb[t][:])

        # ---- layer 2: 3x3 dynamic conv over own half. Static offsets in an
        # If/Else on partition parity (dynamic APs stall the PE sequencer). ----
        ps_c.release()
        ps_y = tc.alloc_tile_pool(name="ps_y", bufs=6, space="PSUM")
        pid = nc.partition_id()
        halfsel = nc.snap(pid % 2, min_val=0, max_val=1)
        y_sb = big.tile([COUT, HALF], F32, tag="ysb")

        def l2_loop(r0):
            for t0, R in L2_TILES:
                n = PW * R
                yp = ps_y.tile([COUT, n], F32, tag="yp", name=f"yp{r0}_{t0}")
                k = 0
                for di in range(3):
                    for dj in range(3):
                        base = HB + (r0 + t0 + di) * PW + dj - 1
                        for t in range(2):
                            nc.tensor.matmul(
                                yp[:],
                                dyn2_sb[t][:, (3 * di + dj) * COUT:
                                           (3 * di + dj + 1) * COUT],
                                hpad[t][:, base:base + n],
                                start=(k == 0), stop=(k == 17))
                            k += 1
                s2 = bass.AP(tensor=yp[:].tensor, offset=yp[:].offset + 1,
                             ap=[list(yp[:].ap[0]), [PW, R], [1, W]])
                nc.vector.tensor_copy(y_sb[:, t0 * W:(t0 + R) * W], s2)
                nc.sync.dma_start(y[:, t0 * W:(t0 + R) * W],
                                  y_sb[:, t0 * W:(t0 + R) * W])

        with tc.If(halfsel < 1) as cmp:
            l2_loop(0)
        with cmp.Else():
            l2_loop(HALF_ROWS)
        ps_y.release()

    nc.finalize()
    return nc


_CACHE = {}


def _get_nc():
    if "nc" not in _CACHE:
        _CACHE["nc"] = _build()
    return _CACHE["nc"]


def _host_weights(fc1_weight, fc1_ce, fc1_gd, fc1_gd2, fc1_ci,
                  fc2_weight, fc2_ce, fc2_gd, fc2_gd2, fc2_ci):
    f = np.float32
    w1 = fc1_weight.reshape(CHID, CIN).astype(f)
    # bd1[c, p*32+o] = fc1_ci[o, c%8] where p = c//8
    bd1 = np.zeros((CIN, CHID), f)
    for c in range(CIN):
        p, g = c // 8, c % 8
        bd1[c, p * 32:(p + 1) * 32] = fc1_ci[:, g]
    # bd2[c, p*2+o] = fc2_ci[o, c%8] where p = c//8
    bd2 = np.zeros((CHID, COUT), f)
    for c in range(CHID):
        p, g = c // 8, c % 8
        bd2[c, p * 2:p * 2 + 2] = fc2_ci[:, g]
    w2t = np.ascontiguousarray(
        fc2_weight.reshape(COUT, CHID, 9).transpose(1, 2, 0).reshape(CHID, 9 * COUT)
    ).astype(f)
    gd2x = np.ascontiguousarray(
        np.repeat(fc2_gd2.T, COUT, axis=1)).astype(f)     # [5, 9*64]
    return {
        "w1t": np.ascontiguousarray(w1.T).astype(f),
        "bd1": bd1,
        "ce1v": np.full((CIN, 1), fc1_ce[0, 0], f),
        "gd1v": np.full((CIN, 1), fc1_gd[0, 0], f),
        "gd21v": np.full((CIN, 1), fc1_gd2[0, 0], f),
        "ones1_64": np.ones((1, CIN), f),
        "ident": np.eye(128, dtype=f),
        "w2t": w2t,
        "bd2": bd2,
        "cewt": np.ascontiguousarray(fc2_ce.T).astype(f),
        "gdt": np.ascontiguousarray(fc2_gd.T).astype(f),
        "gd2x": gd2x,
        "ones5": np.ones((5, 1), f),
        "ones1_128": np.ones((1, 128), f),
    }


def run(inputs, trace=False):
    nc = _get_nc()
    shared = _host_weights(
        inputs["fc1_weight"], inputs["fc1_ce"], inputs["fc1_gd"],
        inputs["fc1_gd2"], inputs["fc1_ci"], inputs["fc2_weight"],
        inputs["fc2_ce"], inputs["fc2_gd"], inputs["fc2_gd2"], inputs["fc2_ci"])
    x = np.asarray(inputs["x"], np.float32)
    in_maps = []
    for core in range(8):
        bi = core // 2
        xb = np.ascontiguousarray(x[bi].reshape(CIN, S))
        in_maps.append({"x64": xb, "x128": xb, **shared})
    res = run_bass_kernel_spmd(nc, in_maps, list(range(8)), trace=trace)
    out = np.empty((B, COUT, H, W), np.float32)
    for core in range(8):
        bi, half = core // 2, core % 2
        out[bi, :, half * HALF_ROWS:(half + 1) * HALF_ROWS, :] = (
            res.results[core]["y"].reshape(COUT, HALF_ROWS, W))
    return out, res


def kernel(**inputs):
    out, _ = run(inputs, trace=False)
    return out
